# revision 16
# baseline (speedup 1.0000x reference)
"""Trainium2 Bass kernel for nn_DeConv2d (stacked per-channel 3-layer MLP).

Reference computation:
  x: [N=8, IC=128, IH=32, IW=32]; per-channel MLP weights stacked along oC=32.
  For each output channel o: a 3-layer MLP (128->256->256->4, relu between)
  applied to every pixel's IC-dim feature vector. Output re-assembled as a
  2x2 "deconv" upsampling: [8, 32, 64, 64].

Strategy:
  - Shard oC across 8 cores (4 channels each); every core gets full x.
  - Feature-major on chip: activations stored [feature, pixel] so every layer
    is out[half] = W[:, half].T @ act with zero transposes (x is naturally
    [IC, pix]; W1/W2/W3 are naturally [K, M]).
  - Layers 1-2 in float32r matmuls (1 cycle/row at N=512, ~2e-4 rel err).
  - Superchunks of 1024 pixels: matmuls fill 512-wide halves of a 2-bank
    [128, 1024] PSUM tile; fused bias+relu runs once per wide tile,
    alternating between the Scalar and Vector engines (fixed per-op cost
    dominates, so fewer/wider elementwise ops win).
  - Layer 3 (256 -> 4 per channel) in bf16 with PE column tiling: the four
    channels' [128 x 4] matmuls run concurrently in separate 32-column array
    groups, accumulating into one shared PSUM bank at partitions 32o..32o+3.
    One fused bias op + one strided DMA emit all 4 channels per chunk.
  - Weights/biases DMA'd before x; output DMAs ride the idle GpSimd queue.
"""

import numpy as np

N, IC, IH, IW = 8, 128, 32, 32
OC, H, KH, KW = 32, 256, 2, 2
NCORES = 8
CPC = OC // NCORES          # channels per core = 4
NPIX = N * IH * IW          # 8192
CH = 512                    # matmul moving free dim (fp32r max)
SC = 1024                   # superchunk (2 matmul chunks, one wide psum tile)
NSC = NPIX // SC            # 8
P = 128
KK = KH * KW                # 4

MM_DTYPE = "float32r"       # "float32r" | "float32"

_COMPILED = None


def _build_bass():
    import concourse.mybir as mybir
    from concourse import bacc
    from concourse.tile import TileContext

    f32 = mybir.dt.float32
    bf16 = mybir.dt.bfloat16
    mm_dt = getattr(mybir.dt, MM_DTYPE)
    AF = mybir.ActivationFunctionType
    ALU = mybir.AluOpType

    nc = bacc.Bacc()
    xr = nc.dram_tensor("xr", [P, NPIX], mm_dt, kind="ExternalInput")
    w1 = nc.dram_tensor("w1", [CPC, IC, H], mm_dt, kind="ExternalInput")
    w2 = nc.dram_tensor("w2", [CPC, H, H], mm_dt, kind="ExternalInput")
    w3 = nc.dram_tensor("w3", [CPC, H, KK], bf16, kind="ExternalInput")
    b1 = nc.dram_tensor("b1", [CPC, H], f32, kind="ExternalInput")
    b2 = nc.dram_tensor("b2", [CPC, H], f32, kind="ExternalInput")
    b3 = nc.dram_tensor("b3", [CPC, KK], f32, kind="ExternalInput")
    out = nc.dram_tensor("out", [CPC, KK, NPIX], f32, kind="ExternalOutput")

    with TileContext(nc) as tc:
        with (
            tc.tile_pool(name="xpool", bufs=1) as xpool,
            tc.tile_pool(name="wpool", bufs=1) as wpool,
            tc.tile_pool(name="fpool", bufs=3) as fpool,
            tc.tile_pool(name="opool", bufs=4) as opool,
            tc.tile_pool(name="pspool", bufs=3, space="PSUM") as pspool,
            tc.tile_pool(name="ps3pool", bufs=1, space="PSUM") as ps3pool,
        ):
            xt = xpool.tile([P, NPIX], mm_dt, tag="x")
            w1t, w2t, w3t, b1t, b2t = [], [], [], [], []
            # b3 scattered across partitions: b3vec[32o + k] = b3[o, k]
            b3vec = wpool.tile([P, 1], f32, tag="b3vec")
            nc.gpsimd.memset(b3vec, 0.0)

            def load_channel_l1(o):
                t1 = wpool.tile([P, H], mm_dt, tag=f"w1_{o}")
                nc.sync.dma_start(out=t1, in_=w1[o])
                w1t.append(t1)
                tb = wpool.tile([P, 2], f32, tag=f"b1_{o}")
                nc.sync.dma_start(out=tb, in_=b1[o].rearrange("(hb p) -> p hb", hb=2))
                b1t.append(tb)

            def load_channel(o):
                # [256, 256] -> [128, 512] with kb chunks side by side
                t2 = wpool.tile([P, 2 * H], mm_dt, tag=f"w2_{o}")
                nc.sync.dma_start(out=t2.rearrange("p (kb h) -> p kb h", kb=2),
                                  in_=w2[o].rearrange("(kb p) h -> p kb h", kb=2))
                w2t.append(t2)
                tb = wpool.tile([P, 2], f32, tag=f"b2_{o}")
                nc.sync.dma_start(out=tb, in_=b2[o].rearrange("(hb p) -> p hb", hb=2))
                b2t.append(tb)
                t3 = wpool.tile([P, 2 * KK], bf16, tag=f"w3_{o}")
                nc.sync.dma_start(out=t3.rearrange("p (kb k) -> p kb k", kb=2),
                                  in_=w3[o].rearrange("(kb p) k -> p kb k", kb=2))
                w3t.append(t3)
                nc.sync.dma_start(out=b3vec[32 * o:32 * o + KK, :],
                                  in_=b3[o].rearrange("(k u) -> k u", u=1))

            # channel-0 layer-1 weights + first x chunks first, so compute starts
            # as early as possible; everything else streams in behind
            load_channel_l1(0)
            nc.sync.dma_start(out=xt[:, 0:CH], in_=xr[:, 0:CH])
            nc.sync.dma_start(out=xt[:, CH:SC], in_=xr[:, CH:SC])
            load_channel(0)
            for o in range(1, CPC):
                load_channel_l1(o)
                load_channel(o)
            for s in range(1, NSC):
                nc.sync.dma_start(out=xt[:, s * SC:(s + 1) * SC],
                                  in_=xr[:, s * SC:(s + 1) * SC])

            # persistent packed layer-3 psum banks; zero once so the rows the
            # packed matmuls never touch stay initialized (uninit PSUM reads
            # are unsafe)
            ps3_slots = []
            for i in range(2):
                t = ps3pool.tile([P, CH], f32, tag=f"ps3_{i}")
                nc.vector.memset(t, 0.0)
                ps3_slots.append(t)

            eltw = 0  # round-robin parity for ACT/DVE balancing

            def bias_relu(dst, src, bias_ap):
                nonlocal eltw
                eltw += 1
                if eltw % 2:
                    nc.scalar.activation(dst, src, AF.Relu, bias=bias_ap)
                else:
                    nc.vector.tensor_scalar(dst, src, bias_ap, 0.0,
                                            op0=ALU.add, op1=ALU.max)

            def bias_add(dst, src, bias_ap):
                nonlocal eltw
                eltw += 1
                if eltw % 2:
                    nc.scalar.activation(dst, src, AF.Identity, bias=bias_ap)
                else:
                    nc.vector.tensor_scalar(dst, src, bias_ap, 0.0,
                                            op0=ALU.add, op1=ALU.add)

            for s in range(NSC):
                xs = xt[:, s * SC:(s + 1) * SC]
                f2s = []
                for o in range(CPC):
                    # ---- layer 1: f1[hb] = relu(W1[:,hb].T @ x + b1[hb]) ----
                    f1 = []
                    for hb in range(2):
                        ps = pspool.tile([P, SC], f32, tag="mm")
                        for c in range(2):
                            nc.tensor.matmul(
                                ps[:, c * CH:(c + 1) * CH],
                                w1t[o][:, hb * P:(hb + 1) * P],
                                xs[:, c * CH:(c + 1) * CH],
                                start=True, stop=True)
                        ft = fpool.tile([P, SC], mm_dt, tag=f"f1_{hb}")
                        bias_relu(ft, ps, b1t[o][:, hb:hb + 1])
                        f1.append(ft)
                    # ---- layer 2: f2[hb] = relu(sum_kb W2[kb][:,hb].T @ f1[kb] + b2) ----
                    f2 = []
                    for hb in range(2):
                        ps = pspool.tile([P, SC], f32, tag="mm")
                        for c in range(2):
                            for kb in range(2):
                                nc.tensor.matmul(
                                    ps[:, c * CH:(c + 1) * CH],
                                    w2t[o][:, kb * H + hb * P:kb * H + (hb + 1) * P],
                                    f1[kb][:, c * CH:(c + 1) * CH],
                                    start=(kb == 0), stop=(kb == 1))
                        ft = fpool.tile([P, SC], bf16, tag=f"f2_{o}_{hb}")
                        bias_relu(ft, ps, b2t[o][:, hb:hb + 1])
                        f2.append(ft)
                    f2s.append(f2)
                # ---- layer 3, packed: 4 channels in separate 32-col PE groups.
                # kb phases are RAW-serial per chunk; interleaving the two
                # chunks' phases hides each phase's drain behind the other.
                for kb in range(2):
                    for c in range(2):
                        for o in range(CPC):
                            nc.tensor.matmul(
                                ps3_slots[c][32 * o:32 * o + KK, :],
                                w3t[o][:, kb * KK:(kb + 1) * KK],
                                f2s[o][kb][:, c * CH:(c + 1) * CH],
                                start=(kb == 0), stop=(kb == 1),
                                tile_position=(0, 32 * o), skip_group_check=True)
                for c in range(2):
                    ot = opool.tile([P, CH], f32, tag="out")
                    bias_add(ot, ps3_slots[c], b3vec)
                    for o in range(CPC):
                        nc.sync.dma_start(
                            out=out[o, :, (2 * s + c) * CH:(2 * s + c + 1) * CH],
                            in_=ot[32 * o:32 * o + KK, :])
    nc.finalize()
    return nc


def _get_compiled():
    global _COMPILED
    if _COMPILED is None:
        _COMPILED = _build_bass()
    return _COMPILED


def make_in_maps(x, W1, b1, W2, b2, W3, b3):
    import ml_dtypes

    x = np.asarray(x, dtype=np.float32)
    # [IC, n*IH*IW] feature-major pixel matrix
    xr = np.ascontiguousarray(
        x.reshape(N, IC, IH * IW).transpose(1, 0, 2).reshape(IC, NPIX))
    in_maps = []
    for c in range(NCORES):
        sl = slice(c * CPC, (c + 1) * CPC)
        in_maps.append({
            "xr": xr,
            "w1": np.ascontiguousarray(np.asarray(W1[sl], dtype=np.float32)),
            "w2": np.ascontiguousarray(np.asarray(W2[sl], dtype=np.float32)),
            "w3": np.ascontiguousarray(
                np.asarray(W3[sl], dtype=np.float32)).astype(ml_dtypes.bfloat16),
            "b1": np.ascontiguousarray(np.asarray(b1[sl], dtype=np.float32)),
            "b2": np.ascontiguousarray(np.asarray(b2[sl], dtype=np.float32)),
            "b3": np.ascontiguousarray(np.asarray(b3[sl], dtype=np.float32)),
        })
    return in_maps


def assemble(results):
    # [oC, kk, npix] -> [oC, kh, kw, n, ih, iw] -> [n, oC, ih, kh, iw, kw]
    all_out = np.concatenate([r["out"] for r in results], axis=0)
    y = all_out.reshape(OC, KH, KW, N, IH, IW)
    y = y.transpose(3, 0, 4, 1, 5, 2).reshape(N, OC, KH * IH, KW * IW)
    return np.ascontiguousarray(y)


def kernel(x, W1, b1, W2, b2, W3, b3):
    from concourse.bass_utils import run_bass_kernel_spmd

    in_maps = make_in_maps(x, W1, b1, W2, b2, W3, b3)
    nc = _get_compiled()
    res = run_bass_kernel_spmd(nc, in_maps, core_ids=list(range(NCORES)))
    return assemble(res.results)


# revision 17
# speedup vs baseline: 1.1741x; 1.1741x over previous
"""Trainium2 Bass kernel for nn_DeConv2d (stacked per-channel 3-layer MLP).

Reference computation:
  x: [N=8, IC=128, IH=32, IW=32]; per-channel MLP weights stacked along oC=32.
  For each output channel o: a 3-layer MLP (128->256->256->4, relu between)
  applied to every pixel's IC-dim feature vector. Output re-assembled as a
  2x2 "deconv" upsampling: [8, 32, 64, 64].

Strategy:
  - Shard oC across 8 cores (4 channels each); every core gets full x.
  - Feature-major on chip: activations stored [feature, pixel] so every layer
    is out[half] = W[:, half].T @ act with zero transposes (x is naturally
    [IC, pix]; W1/W2/W3 are naturally [K, M]).
  - Layers 1-2 in float32r matmuls (1 cycle/row at N=512, ~2e-4 rel err).
  - Superchunks of 1024 pixels: matmuls fill 512-wide halves of a 2-bank
    [128, 1024] PSUM tile; fused bias+relu runs once per wide tile,
    alternating between the Scalar and Vector engines (fixed per-op cost
    dominates, so fewer/wider elementwise ops win).
  - Layer 3 (256 -> 4 per channel) in bf16 with PE column tiling: the four
    channels' [128 x 4] matmuls run concurrently in separate 32-column array
    groups, accumulating into one shared PSUM bank at partitions 32o..32o+3.
    One fused bias op + one strided DMA emit all 4 channels per chunk.
  - Weights/biases DMA'd before x; output DMAs ride the idle GpSimd queue.
"""

import numpy as np

N, IC, IH, IW = 8, 128, 32, 32
OC, H, KH, KW = 32, 256, 2, 2
NCORES = 8
CPC = OC // NCORES          # channels per core = 4
NPIX = N * IH * IW          # 8192
CH = 512                    # matmul moving free dim (fp32r max)
SC = 1024                   # superchunk (2 matmul chunks, one wide psum tile)
NSC = NPIX // SC            # 8
P = 128
KK = KH * KW                # 4

MM_DTYPE = "float32r"       # "float32r" | "float32"

_COMPILED = None


def _build_bass():
    import concourse.mybir as mybir
    from concourse import bacc
    from concourse.tile import TileContext

    f32 = mybir.dt.float32
    bf16 = mybir.dt.bfloat16
    mm_dt = getattr(mybir.dt, MM_DTYPE)
    AF = mybir.ActivationFunctionType
    ALU = mybir.AluOpType

    nc = bacc.Bacc()
    xr = nc.dram_tensor("xr", [P, NPIX], mm_dt, kind="ExternalInput")
    w1 = nc.dram_tensor("w1", [CPC, IC, H], mm_dt, kind="ExternalInput")
    w2 = nc.dram_tensor("w2", [CPC, H, H], mm_dt, kind="ExternalInput")
    w3 = nc.dram_tensor("w3", [CPC, H, KK], bf16, kind="ExternalInput")
    b1 = nc.dram_tensor("b1", [CPC, H], f32, kind="ExternalInput")
    b2 = nc.dram_tensor("b2", [CPC, H], f32, kind="ExternalInput")
    b3 = nc.dram_tensor("b3", [CPC, KK], f32, kind="ExternalInput")
    out = nc.dram_tensor("out", [CPC, KK, NPIX], f32, kind="ExternalOutput")

    with TileContext(nc) as tc:
        with (
            tc.tile_pool(name="xpool", bufs=1) as xpool,
            tc.tile_pool(name="wpool", bufs=1) as wpool,
            tc.tile_pool(name="fpool", bufs=3) as fpool,
            tc.tile_pool(name="opool", bufs=4) as opool,
            tc.tile_pool(name="pspool", bufs=3, space="PSUM") as pspool,
            tc.tile_pool(name="ps3pool", bufs=1, space="PSUM") as ps3pool,
        ):
            xt = xpool.tile([P, NPIX], mm_dt, tag="x")
            w1t, w2t, w3t, b1t, b2t = [], [], [], [], []
            # b3 scattered across partitions: b3vec[32o + k] = b3[o, k]
            b3vec = wpool.tile([P, 1], f32, tag="b3vec")
            nc.gpsimd.memset(b3vec, 0.0)

            def load_channel_l1(o):
                t1 = wpool.tile([P, H], mm_dt, tag=f"w1_{o}")
                nc.sync.dma_start(out=t1, in_=w1[o])
                w1t.append(t1)
                tb = wpool.tile([P, 2], f32, tag=f"b1_{o}")
                nc.sync.dma_start(out=tb, in_=b1[o].rearrange("(hb p) -> p hb", hb=2))
                b1t.append(tb)

            def load_channel(o):
                # [256, 256] -> [128, 512] with kb chunks side by side
                t2 = wpool.tile([P, 2 * H], mm_dt, tag=f"w2_{o}")
                nc.sync.dma_start(out=t2.rearrange("p (kb h) -> p kb h", kb=2),
                                  in_=w2[o].rearrange("(kb p) h -> p kb h", kb=2))
                w2t.append(t2)
                tb = wpool.tile([P, 2], f32, tag=f"b2_{o}")
                nc.sync.dma_start(out=tb, in_=b2[o].rearrange("(hb p) -> p hb", hb=2))
                b2t.append(tb)
                t3 = wpool.tile([P, 2 * KK], bf16, tag=f"w3_{o}")
                nc.sync.dma_start(out=t3.rearrange("p (kb k) -> p kb k", kb=2),
                                  in_=w3[o].rearrange("(kb p) k -> p kb k", kb=2))
                w3t.append(t3)
                nc.sync.dma_start(out=b3vec[32 * o:32 * o + KK, :],
                                  in_=b3[o].rearrange("(k u) -> k u", u=1))

            # channel-0 layer-1 weights + first x chunks first, so compute starts
            # as early as possible; everything else streams in behind
            load_channel_l1(0)
            nc.sync.dma_start(out=xt[:, 0:CH], in_=xr[:, 0:CH])
            nc.sync.dma_start(out=xt[:, CH:SC], in_=xr[:, CH:SC])
            load_channel(0)
            for o in range(1, CPC):
                load_channel_l1(o)
                load_channel(o)
            for s in range(1, NSC):
                nc.sync.dma_start(out=xt[:, s * SC:(s + 1) * SC],
                                  in_=xr[:, s * SC:(s + 1) * SC])

            # persistent packed layer-3 psum banks; zero once so the rows the
            # packed matmuls never touch stay initialized (uninit PSUM reads
            # are unsafe)
            ps3_slots = []
            for i in range(2):
                t = ps3pool.tile([P, CH], f32, tag=f"ps3_{i}")
                nc.vector.memset(t, 0.0)
                ps3_slots.append(t)

            eltw = 0  # round-robin parity for ACT/DVE balancing

            def bias_relu(dst, src, bias_ap):
                nonlocal eltw
                eltw += 1
                if eltw % 2:
                    nc.scalar.activation(dst, src, AF.Relu, bias=bias_ap)
                else:
                    nc.vector.tensor_scalar(dst, src, bias_ap, 0.0,
                                            op0=ALU.add, op1=ALU.max)

            def bias_add(dst, src, bias_ap):
                nonlocal eltw
                eltw += 1
                if eltw % 2:
                    nc.scalar.activation(dst, src, AF.Identity, bias=bias_ap)
                else:
                    nc.vector.tensor_scalar(dst, src, bias_ap, 0.0,
                                            op0=ALU.add, op1=ALU.add)

            def layer1(s, o):
                xs = xt[:, s * SC:(s + 1) * SC]
                f1 = []
                for hb in range(2):
                    ps = pspool.tile([P, SC], f32, tag="mm")
                    for c in range(2):
                        nc.tensor.matmul(
                            ps[:, c * CH:(c + 1) * CH],
                            w1t[o][:, hb * P:(hb + 1) * P],
                            xs[:, c * CH:(c + 1) * CH],
                            start=True, stop=True)
                    ft = fpool.tile([P, SC], mm_dt, tag=f"f1_{o % 2}_{hb}")
                    bias_relu(ft, ps, b1t[o][:, hb:hb + 1])
                    f1.append(ft)
                return f1

            def layer2(s, o, f1):
                f2 = []
                for hb in range(2):
                    ps = pspool.tile([P, SC], f32, tag="mm")
                    for c in range(2):
                        for kb in range(2):
                            nc.tensor.matmul(
                                ps[:, c * CH:(c + 1) * CH],
                                w2t[o][:, kb * H + hb * P:kb * H + (hb + 1) * P],
                                f1[kb][:, c * CH:(c + 1) * CH],
                                start=(kb == 0), stop=(kb == 1))
                    ft = fpool.tile([P, SC], bf16, tag=f"f2_{o}_{hb}")
                    bias_relu(ft, ps, b2t[o][:, hb:hb + 1])
                    f2.append(ft)
                return f2

            for s in range(NSC):
                f2s = [None] * CPC
                # channel pairs: emit both channels' layer-1 matmuls before
                # either channel's layer-2, so PE has ready work while the
                # layer-1 relus drain
                for op in range(0, CPC, 2):
                    f1a = layer1(s, op)
                    f1b = layer1(s, op + 1)
                    f2s[op] = layer2(s, op, f1a)
                    f2s[op + 1] = layer2(s, op + 1, f1b)
                # ---- layer 3, packed: 4 channels in separate 32-col PE groups.
                # kb phases are RAW-serial per chunk; interleaving the two
                # chunks' phases hides each phase's drain behind the other.
                for kb in range(2):
                    for c in range(2):
                        for o in range(CPC):
                            nc.tensor.matmul(
                                ps3_slots[c][32 * o:32 * o + KK, :],
                                w3t[o][:, kb * KK:(kb + 1) * KK],
                                f2s[o][kb][:, c * CH:(c + 1) * CH],
                                start=(kb == 0), stop=(kb == 1),
                                tile_position=(0, 32 * o), skip_group_check=True)
                for c in range(2):
                    ot = opool.tile([P, CH], f32, tag="out")
                    bias_add(ot, ps3_slots[c], b3vec)
                    for o in range(CPC):
                        nc.sync.dma_start(
                            out=out[o, :, (2 * s + c) * CH:(2 * s + c + 1) * CH],
                            in_=ot[32 * o:32 * o + KK, :])
    nc.finalize()
    return nc


def _get_compiled():
    global _COMPILED
    if _COMPILED is None:
        _COMPILED = _build_bass()
    return _COMPILED


def make_in_maps(x, W1, b1, W2, b2, W3, b3):
    import ml_dtypes

    x = np.asarray(x, dtype=np.float32)
    # [IC, n*IH*IW] feature-major pixel matrix
    xr = np.ascontiguousarray(
        x.reshape(N, IC, IH * IW).transpose(1, 0, 2).reshape(IC, NPIX))
    in_maps = []
    for c in range(NCORES):
        sl = slice(c * CPC, (c + 1) * CPC)
        in_maps.append({
            "xr": xr,
            "w1": np.ascontiguousarray(np.asarray(W1[sl], dtype=np.float32)),
            "w2": np.ascontiguousarray(np.asarray(W2[sl], dtype=np.float32)),
            "w3": np.ascontiguousarray(
                np.asarray(W3[sl], dtype=np.float32)).astype(ml_dtypes.bfloat16),
            "b1": np.ascontiguousarray(np.asarray(b1[sl], dtype=np.float32)),
            "b2": np.ascontiguousarray(np.asarray(b2[sl], dtype=np.float32)),
            "b3": np.ascontiguousarray(np.asarray(b3[sl], dtype=np.float32)),
        })
    return in_maps


def assemble(results):
    # [oC, kk, npix] -> [oC, kh, kw, n, ih, iw] -> [n, oC, ih, kh, iw, kw]
    all_out = np.concatenate([r["out"] for r in results], axis=0)
    y = all_out.reshape(OC, KH, KW, N, IH, IW)
    y = y.transpose(3, 0, 4, 1, 5, 2).reshape(N, OC, KH * IH, KW * IW)
    return np.ascontiguousarray(y)


def kernel(x, W1, b1, W2, b2, W3, b3):
    from concourse.bass_utils import run_bass_kernel_spmd

    in_maps = make_in_maps(x, W1, b1, W2, b2, W3, b3)
    nc = _get_compiled()
    res = run_bass_kernel_spmd(nc, in_maps, core_ids=list(range(NCORES)))
    return assemble(res.results)


# revision 18
# speedup vs baseline: 1.1770x; 1.0025x over previous
"""Trainium2 Bass kernel for nn_DeConv2d (stacked per-channel 3-layer MLP).

Reference computation:
  x: [N=8, IC=128, IH=32, IW=32]; per-channel MLP weights stacked along oC=32.
  For each output channel o: a 3-layer MLP (128->256->256->4, relu between)
  applied to every pixel's IC-dim feature vector. Output re-assembled as a
  2x2 "deconv" upsampling: [8, 32, 64, 64].

Strategy:
  - Shard oC across 8 cores (4 channels each); every core gets full x.
  - Feature-major on chip: activations stored [feature, pixel] so every layer
    is out[half] = W[:, half].T @ act with zero transposes (x is naturally
    [IC, pix]; W1/W2/W3 are naturally [K, M]).
  - Layers 1-2 in float32r matmuls (1 cycle/row at N=512, ~2e-4 rel err).
  - Superchunks of 1024 pixels: matmuls fill 512-wide halves of a 2-bank
    [128, 1024] PSUM tile; fused bias+relu runs once per wide tile,
    alternating between the Scalar and Vector engines (fixed per-op cost
    dominates, so fewer/wider elementwise ops win).
  - Layer 3 (256 -> 4 per channel) in bf16 with PE column tiling: the four
    channels' [128 x 4] matmuls run concurrently in separate 32-column array
    groups, accumulating into one shared PSUM bank at partitions 32o..32o+3.
    One fused bias op + one strided DMA emit all 4 channels per chunk.
  - Weights/biases DMA'd before x; output DMAs ride the idle GpSimd queue.
"""

import numpy as np

N, IC, IH, IW = 8, 128, 32, 32
OC, H, KH, KW = 32, 256, 2, 2
NCORES = 8
CPC = OC // NCORES          # channels per core = 4
NPIX = N * IH * IW          # 8192
CH = 512                    # matmul moving free dim (fp32r max)
SC = 1024                   # superchunk (2 matmul chunks, one wide psum tile)
NSC = NPIX // SC            # 8
P = 128
KK = KH * KW                # 4

MM_DTYPE = "float32r"       # "float32r" | "float32"

_COMPILED = None


def _build_bass():
    import concourse.mybir as mybir
    from concourse import bacc
    from concourse.tile import TileContext

    f32 = mybir.dt.float32
    bf16 = mybir.dt.bfloat16
    mm_dt = getattr(mybir.dt, MM_DTYPE)
    AF = mybir.ActivationFunctionType
    ALU = mybir.AluOpType

    nc = bacc.Bacc()
    xr = nc.dram_tensor("xr", [P, NPIX], mm_dt, kind="ExternalInput")
    w1 = nc.dram_tensor("w1", [CPC, IC, H], mm_dt, kind="ExternalInput")
    w2 = nc.dram_tensor("w2", [CPC, H, H], mm_dt, kind="ExternalInput")
    w3 = nc.dram_tensor("w3", [CPC, H, KK], bf16, kind="ExternalInput")
    b1 = nc.dram_tensor("b1", [CPC, H], f32, kind="ExternalInput")
    b2 = nc.dram_tensor("b2", [CPC, H], f32, kind="ExternalInput")
    b3 = nc.dram_tensor("b3", [CPC, KK], f32, kind="ExternalInput")
    out = nc.dram_tensor("out", [CPC, KK, NPIX], f32, kind="ExternalOutput")

    with TileContext(nc) as tc:
        with (
            tc.tile_pool(name="xpool", bufs=1) as xpool,
            tc.tile_pool(name="wpool", bufs=1) as wpool,
            tc.tile_pool(name="fpool", bufs=3) as fpool,
            tc.tile_pool(name="opool", bufs=4) as opool,
            tc.tile_pool(name="pspool", bufs=3, space="PSUM") as pspool,
            tc.tile_pool(name="ps3pool", bufs=1, space="PSUM") as ps3pool,
        ):
            xt = xpool.tile([P, NPIX], mm_dt, tag="x")
            w1t, w2t, w3t, b1t, b2t = [], [], [], [], []
            # b3 scattered across partitions: b3vec[32o + k] = b3[o, k]
            b3vec = wpool.tile([P, 1], f32, tag="b3vec")
            nc.gpsimd.memset(b3vec, 0.0)

            def load_channel_l1(o):
                t1 = wpool.tile([P, H], mm_dt, tag=f"w1_{o}")
                nc.sync.dma_start(out=t1, in_=w1[o])
                w1t.append(t1)
                tb = wpool.tile([P, 2], f32, tag=f"b1_{o}")
                nc.sync.dma_start(out=tb, in_=b1[o].rearrange("(hb p) -> p hb", hb=2))
                b1t.append(tb)

            def load_channel(o):
                # [256, 256] -> [128, 512] with kb chunks side by side
                t2 = wpool.tile([P, 2 * H], mm_dt, tag=f"w2_{o}")
                nc.sync.dma_start(out=t2.rearrange("p (kb h) -> p kb h", kb=2),
                                  in_=w2[o].rearrange("(kb p) h -> p kb h", kb=2))
                w2t.append(t2)
                tb = wpool.tile([P, 2], f32, tag=f"b2_{o}")
                nc.sync.dma_start(out=tb, in_=b2[o].rearrange("(hb p) -> p hb", hb=2))
                b2t.append(tb)
                t3 = wpool.tile([P, 2 * KK], bf16, tag=f"w3_{o}")
                nc.sync.dma_start(out=t3.rearrange("p (kb k) -> p kb k", kb=2),
                                  in_=w3[o].rearrange("(kb p) k -> p kb k", kb=2))
                w3t.append(t3)
                nc.sync.dma_start(out=b3vec[32 * o:32 * o + KK, :],
                                  in_=b3[o].rearrange("(k u) -> k u", u=1))

            # channel-0 layer-1 weights + first x chunks first, so compute starts
            # as early as possible; everything else streams in behind
            t1 = wpool.tile([P, H], mm_dt, tag="w1_0")
            nc.sync.dma_start(out=t1, in_=w1[0])
            w1t.append(t1)
            nc.sync.dma_start(out=xt[:, 0:CH], in_=xr[:, 0:CH])
            tb = wpool.tile([P, 2], f32, tag="b1_0")
            nc.sync.dma_start(out=tb, in_=b1[0].rearrange("(hb p) -> p hb", hb=2))
            b1t.append(tb)
            nc.sync.dma_start(out=xt[:, CH:SC], in_=xr[:, CH:SC])
            load_channel_l1(1)
            load_channel(0)
            for o in range(2, CPC):
                load_channel_l1(o)
            load_channel(1)
            for o in range(2, CPC):
                load_channel(o)
            for s in range(1, NSC):
                nc.sync.dma_start(out=xt[:, s * SC:(s + 1) * SC],
                                  in_=xr[:, s * SC:(s + 1) * SC])

            # persistent packed layer-3 psum banks; zero once so the rows the
            # packed matmuls never touch stay initialized (uninit PSUM reads
            # are unsafe)
            ps3_slots = []
            for i in range(2):
                t = ps3pool.tile([P, CH], f32, tag=f"ps3_{i}")
                nc.vector.memset(t, 0.0)
                ps3_slots.append(t)

            eltw = 0  # round-robin parity for ACT/DVE balancing

            def bias_relu(dst, src, bias_ap):
                nonlocal eltw
                eltw += 1
                if eltw % 2:
                    nc.scalar.activation(dst, src, AF.Relu, bias=bias_ap)
                else:
                    nc.vector.tensor_scalar(dst, src, bias_ap, 0.0,
                                            op0=ALU.add, op1=ALU.max)

            def bias_add(dst, src, bias_ap):
                nonlocal eltw
                eltw += 1
                if eltw % 2:
                    nc.scalar.activation(dst, src, AF.Identity, bias=bias_ap)
                else:
                    nc.vector.tensor_scalar(dst, src, bias_ap, 0.0,
                                            op0=ALU.add, op1=ALU.add)

            def layer1(s, o):
                xs = xt[:, s * SC:(s + 1) * SC]
                f1 = []
                for hb in range(2):
                    ps = pspool.tile([P, SC], f32, tag="mm")
                    for c in range(2):
                        nc.tensor.matmul(
                            ps[:, c * CH:(c + 1) * CH],
                            w1t[o][:, hb * P:(hb + 1) * P],
                            xs[:, c * CH:(c + 1) * CH],
                            start=True, stop=True)
                    ft = fpool.tile([P, SC], mm_dt, tag=f"f1_{o % 2}_{hb}")
                    bias_relu(ft, ps, b1t[o][:, hb:hb + 1])
                    f1.append(ft)
                return f1

            def layer2(s, o, f1):
                f2 = []
                for hb in range(2):
                    ps = pspool.tile([P, SC], f32, tag="mm")
                    for c in range(2):
                        for kb in range(2):
                            nc.tensor.matmul(
                                ps[:, c * CH:(c + 1) * CH],
                                w2t[o][:, kb * H + hb * P:kb * H + (hb + 1) * P],
                                f1[kb][:, c * CH:(c + 1) * CH],
                                start=(kb == 0), stop=(kb == 1))
                    ft = fpool.tile([P, SC], bf16, tag=f"f2_{o}_{hb}")
                    bias_relu(ft, ps, b2t[o][:, hb:hb + 1])
                    f2.append(ft)
                return f2

            for s in range(NSC):
                f2s = [None] * CPC
                # channel pairs: emit both channels' layer-1 matmuls before
                # either channel's layer-2, so PE has ready work while the
                # layer-1 relus drain
                for op in range(0, CPC, 2):
                    f1a = layer1(s, op)
                    f1b = layer1(s, op + 1)
                    f2s[op] = layer2(s, op, f1a)
                    f2s[op + 1] = layer2(s, op + 1, f1b)
                # ---- layer 3, packed: 4 channels in separate 32-col PE groups.
                # kb phases are RAW-serial per chunk; interleaving the two
                # chunks' phases hides each phase's drain behind the other.
                for kb in range(2):
                    for c in range(2):
                        for o in range(CPC):
                            nc.tensor.matmul(
                                ps3_slots[c][32 * o:32 * o + KK, :],
                                w3t[o][:, kb * KK:(kb + 1) * KK],
                                f2s[o][kb][:, c * CH:(c + 1) * CH],
                                start=(kb == 0), stop=(kb == 1),
                                tile_position=(0, 32 * o), skip_group_check=True)
                for c in range(2):
                    ot = opool.tile([P, CH], f32, tag="out")
                    bias_add(ot, ps3_slots[c], b3vec)
                    for o in range(CPC):
                        nc.sync.dma_start(
                            out=out[o, :, (2 * s + c) * CH:(2 * s + c + 1) * CH],
                            in_=ot[32 * o:32 * o + KK, :])
    nc.finalize()
    return nc


def _get_compiled():
    global _COMPILED
    if _COMPILED is None:
        _COMPILED = _build_bass()
    return _COMPILED


def make_in_maps(x, W1, b1, W2, b2, W3, b3):
    import ml_dtypes

    x = np.asarray(x, dtype=np.float32)
    # [IC, n*IH*IW] feature-major pixel matrix
    xr = np.ascontiguousarray(
        x.reshape(N, IC, IH * IW).transpose(1, 0, 2).reshape(IC, NPIX))
    in_maps = []
    for c in range(NCORES):
        sl = slice(c * CPC, (c + 1) * CPC)
        in_maps.append({
            "xr": xr,
            "w1": np.ascontiguousarray(np.asarray(W1[sl], dtype=np.float32)),
            "w2": np.ascontiguousarray(np.asarray(W2[sl], dtype=np.float32)),
            "w3": np.ascontiguousarray(
                np.asarray(W3[sl], dtype=np.float32)).astype(ml_dtypes.bfloat16),
            "b1": np.ascontiguousarray(np.asarray(b1[sl], dtype=np.float32)),
            "b2": np.ascontiguousarray(np.asarray(b2[sl], dtype=np.float32)),
            "b3": np.ascontiguousarray(np.asarray(b3[sl], dtype=np.float32)),
        })
    return in_maps


def assemble(results):
    # [oC, kk, npix] -> [oC, kh, kw, n, ih, iw] -> [n, oC, ih, kh, iw, kw]
    all_out = np.concatenate([r["out"] for r in results], axis=0)
    y = all_out.reshape(OC, KH, KW, N, IH, IW)
    y = y.transpose(3, 0, 4, 1, 5, 2).reshape(N, OC, KH * IH, KW * IW)
    return np.ascontiguousarray(y)


def kernel(x, W1, b1, W2, b2, W3, b3):
    from concourse.bass_utils import run_bass_kernel_spmd

    in_maps = make_in_maps(x, W1, b1, W2, b2, W3, b3)
    nc = _get_compiled()
    res = run_bass_kernel_spmd(nc, in_maps, core_ids=list(range(NCORES)))
    return assemble(res.results)


# revision 19
# speedup vs baseline: 1.2063x; 1.0249x over previous
"""Trainium2 Bass kernel for nn_DeConv2d (stacked per-channel 3-layer MLP).

Reference computation:
  x: [N=8, IC=128, IH=32, IW=32]; per-channel MLP weights stacked along oC=32.
  For each output channel o: a 3-layer MLP (128->256->256->4, relu between)
  applied to every pixel's IC-dim feature vector. Output re-assembled as a
  2x2 "deconv" upsampling: [8, 32, 64, 64].

Strategy:
  - Shard oC across 8 cores (4 channels each); every core gets full x.
  - Feature-major on chip: activations stored [feature, pixel] so every layer
    is out[half] = W[:, half].T @ act with zero transposes (x is naturally
    [IC, pix]; W1/W2/W3 are naturally [K, M]).
  - Layers 1-2 in float32r matmuls (1 cycle/row at N=512, ~2e-4 rel err).
  - Superchunks of 1024 pixels: matmuls fill 512-wide halves of a 2-bank
    [128, 1024] PSUM tile; fused bias+relu runs once per wide tile,
    alternating between the Scalar and Vector engines (fixed per-op cost
    dominates, so fewer/wider elementwise ops win).
  - Layer 3 (256 -> 4 per channel) in bf16 with PE column tiling: the four
    channels' [128 x 4] matmuls run concurrently in separate 32-column array
    groups, accumulating into one shared PSUM bank at partitions 32o..32o+3.
    One fused bias op + one strided DMA emit all 4 channels per chunk.
  - Weights/biases DMA'd before x; output DMAs ride the idle GpSimd queue.
"""

import numpy as np

N, IC, IH, IW = 8, 128, 32, 32
OC, H, KH, KW = 32, 256, 2, 2
NCORES = 8
CPC = OC // NCORES          # channels per core = 4
NPIX = N * IH * IW          # 8192
CH = 512                    # matmul moving free dim (fp32r max)
SC = 1024                   # superchunk (2 matmul chunks, one wide psum tile)
NSC = NPIX // SC            # 8
P = 128
KK = KH * KW                # 4

MM_DTYPE = "float32r"       # "float32r" | "float32"

_COMPILED = None


def _build_bass():
    import concourse.mybir as mybir
    from concourse import bacc
    from concourse.tile import TileContext

    f32 = mybir.dt.float32
    bf16 = mybir.dt.bfloat16
    mm_dt = getattr(mybir.dt, MM_DTYPE)
    AF = mybir.ActivationFunctionType
    ALU = mybir.AluOpType

    nc = bacc.Bacc()
    xr = nc.dram_tensor("xr", [P, NPIX], mm_dt, kind="ExternalInput")
    w1 = nc.dram_tensor("w1", [CPC, IC, H], mm_dt, kind="ExternalInput")
    w2 = nc.dram_tensor("w2", [CPC, H, H], mm_dt, kind="ExternalInput")
    w3 = nc.dram_tensor("w3", [CPC, H, KK], bf16, kind="ExternalInput")
    b1 = nc.dram_tensor("b1", [CPC, H], f32, kind="ExternalInput")
    b2 = nc.dram_tensor("b2", [CPC, H], f32, kind="ExternalInput")
    b3 = nc.dram_tensor("b3", [CPC, KK], f32, kind="ExternalInput")
    out = nc.dram_tensor("out", [CPC, KK, NPIX], f32, kind="ExternalOutput")

    with TileContext(nc) as tc:
        with (
            tc.tile_pool(name="xpool", bufs=1) as xpool,
            tc.tile_pool(name="wpool", bufs=1) as wpool,
            tc.tile_pool(name="fpool", bufs=3) as fpool,
            tc.tile_pool(name="opool", bufs=4) as opool,
            tc.tile_pool(name="pspool", bufs=3, space="PSUM") as pspool,
            tc.tile_pool(name="ps3pool", bufs=1, space="PSUM") as ps3pool,
        ):
            xt = xpool.tile([P, NPIX], mm_dt, tag="x")
            w1t, w2t, w3t, b1t, b2t = [], [], [], [], []
            # b3 scattered across partitions: b3vec[32o + k] = b3[o, k]
            b3vec = wpool.tile([P, 1], f32, tag="b3vec")
            nc.gpsimd.memset(b3vec, 0.0)

            def load_channel_l1(o):
                t1 = wpool.tile([P, H], mm_dt, tag=f"w1_{o}")
                nc.sync.dma_start(out=t1, in_=w1[o])
                w1t.append(t1)
                tb = wpool.tile([P, 2], f32, tag=f"b1_{o}")
                nc.sync.dma_start(out=tb, in_=b1[o].rearrange("(hb p) -> p hb", hb=2))
                b1t.append(tb)

            def load_channel(o):
                # [256, 256] -> [128, 512] with kb chunks side by side
                t2 = wpool.tile([P, 2 * H], mm_dt, tag=f"w2_{o}")
                nc.sync.dma_start(out=t2.rearrange("p (kb h) -> p kb h", kb=2),
                                  in_=w2[o].rearrange("(kb p) h -> p kb h", kb=2))
                w2t.append(t2)
                tb = wpool.tile([P, 2], f32, tag=f"b2_{o}")
                nc.sync.dma_start(out=tb, in_=b2[o].rearrange("(hb p) -> p hb", hb=2))
                b2t.append(tb)
                t3 = wpool.tile([P, 2 * KK], bf16, tag=f"w3_{o}")
                nc.sync.dma_start(out=t3.rearrange("p (kb k) -> p kb k", kb=2),
                                  in_=w3[o].rearrange("(kb p) k -> p kb k", kb=2))
                w3t.append(t3)
                nc.sync.dma_start(out=b3vec[32 * o:32 * o + KK, :],
                                  in_=b3[o].rearrange("(k u) -> k u", u=1))

            # channel-0 layer-1 weights + first x chunks first, so compute starts
            # as early as possible; everything else streams in behind
            t1 = wpool.tile([P, H], mm_dt, tag="w1_0")
            nc.sync.dma_start(out=t1, in_=w1[0])
            w1t.append(t1)
            nc.sync.dma_start(out=xt[:, 0:CH], in_=xr[:, 0:CH])
            tb = wpool.tile([P, 2], f32, tag="b1_0")
            nc.sync.dma_start(out=tb, in_=b1[0].rearrange("(hb p) -> p hb", hb=2))
            b1t.append(tb)
            nc.sync.dma_start(out=xt[:, CH:SC], in_=xr[:, CH:SC])
            load_channel_l1(1)
            load_channel(0)
            for o in range(2, CPC):
                load_channel_l1(o)
            load_channel(1)
            for o in range(2, CPC):
                load_channel(o)
            for s in range(1, NSC):
                nc.sync.dma_start(out=xt[:, s * SC:(s + 1) * SC],
                                  in_=xr[:, s * SC:(s + 1) * SC])

            # persistent packed layer-3 psum banks; zero once so the rows the
            # packed matmuls never touch stay initialized (uninit PSUM reads
            # are unsafe)
            ps3_slots = []
            for i in range(2):
                t = ps3pool.tile([P, CH], f32, tag=f"ps3_{i}")
                nc.vector.memset(t, 0.0)
                ps3_slots.append(t)

            eltw = 0  # round-robin parity for ACT/DVE balancing

            def bias_relu(dst, src, bias_ap):
                nonlocal eltw
                eltw += 1
                if eltw % 2:
                    nc.scalar.activation(dst, src, AF.Relu, bias=bias_ap)
                else:
                    nc.vector.tensor_scalar(dst, src, bias_ap, 0.0,
                                            op0=ALU.add, op1=ALU.max)

            def bias_add(dst, src, bias_ap):
                nonlocal eltw
                eltw += 1
                if eltw % 2:
                    nc.scalar.activation(dst, src, AF.Identity, bias=bias_ap)
                else:
                    nc.vector.tensor_scalar(dst, src, bias_ap, 0.0,
                                            op0=ALU.add, op1=ALU.add)

            def layer1(s, o):
                xs = xt[:, s * SC:(s + 1) * SC]
                f1 = []
                for hb in range(2):
                    ps = pspool.tile([P, SC], f32, tag="mm")
                    for c in range(2):
                        nc.tensor.matmul(
                            ps[:, c * CH:(c + 1) * CH],
                            w1t[o][:, hb * P:(hb + 1) * P],
                            xs[:, c * CH:(c + 1) * CH],
                            start=True, stop=True)
                    ft = fpool.tile([P, SC], mm_dt, tag=f"f1_{o % 2}_{hb}")
                    bias_relu(ft, ps, b1t[o][:, hb:hb + 1])
                    f1.append(ft)
                return f1

            def layer2(s, o, f1):
                f2 = []
                for hb in range(2):
                    ps = pspool.tile([P, SC], f32, tag="mm")
                    for c in range(2):
                        for kb in range(2):
                            nc.tensor.matmul(
                                ps[:, c * CH:(c + 1) * CH],
                                w2t[o][:, kb * H + hb * P:kb * H + (hb + 1) * P],
                                f1[kb][:, c * CH:(c + 1) * CH],
                                start=(kb == 0), stop=(kb == 1))
                    ft = fpool.tile([P, SC], bf16, tag=f"f2_{o}_{hb}")
                    bias_relu(ft, ps, b2t[o][:, hb:hb + 1])
                    f2.append(ft)
                return f2

            def l3_phase(s, kb, c, f2s):
                for o in range(CPC):
                    nc.tensor.matmul(
                        ps3_slots[c][32 * o:32 * o + KK, :],
                        w3t[o][:, kb * KK:(kb + 1) * KK],
                        f2s[o][kb][:, c * CH:(c + 1) * CH],
                        start=(kb == 0), stop=(kb == 1),
                        tile_position=(0, 32 * o), skip_group_check=True)

            def l3_out(s, f2s):
                for c in range(2):
                    ot = opool.tile([P, CH], f32, tag="out")
                    bias_add(ot, ps3_slots[c], b3vec)
                    for o in range(CPC):
                        nc.sync.dma_start(
                            out=out[o, :, (2 * s + c) * CH:(2 * s + c + 1) * CH],
                            in_=ot[32 * o:32 * o + KK, :])

            # Layer 3 of superchunk s-1 is emitted during superchunk s, its
            # four RAW-serial packed phases interleaved with s's layer-1
            # matmuls so the PE always has streaming work during each
            # phase's drain/weight-load bubble.
            prev = None  # (s-1, f2s of s-1)
            for s in range(NSC):
                f2s = [None] * CPC
                f1a = layer1(s, 0)
                if prev is not None:
                    l3_phase(prev[0], 0, 0, prev[1])
                f1b = layer1(s, 1)
                if prev is not None:
                    l3_phase(prev[0], 0, 1, prev[1])
                f2s[0] = layer2(s, 0, f1a)
                if prev is not None:
                    l3_phase(prev[0], 1, 0, prev[1])
                f2s[1] = layer2(s, 1, f1b)
                if prev is not None:
                    l3_phase(prev[0], 1, 1, prev[1])
                    l3_out(prev[0], prev[1])
                f1a = layer1(s, 2)
                f1b = layer1(s, 3)
                f2s[2] = layer2(s, 2, f1a)
                f2s[3] = layer2(s, 3, f1b)
                prev = (s, f2s)
            for kb in range(2):
                for c in range(2):
                    l3_phase(prev[0], kb, c, prev[1])
            l3_out(prev[0], prev[1])
    nc.finalize()
    return nc


def _get_compiled():
    global _COMPILED
    if _COMPILED is None:
        _COMPILED = _build_bass()
    return _COMPILED


def make_in_maps(x, W1, b1, W2, b2, W3, b3):
    import ml_dtypes

    x = np.asarray(x, dtype=np.float32)
    # [IC, n*IH*IW] feature-major pixel matrix
    xr = np.ascontiguousarray(
        x.reshape(N, IC, IH * IW).transpose(1, 0, 2).reshape(IC, NPIX))
    in_maps = []
    for c in range(NCORES):
        sl = slice(c * CPC, (c + 1) * CPC)
        in_maps.append({
            "xr": xr,
            "w1": np.ascontiguousarray(np.asarray(W1[sl], dtype=np.float32)),
            "w2": np.ascontiguousarray(np.asarray(W2[sl], dtype=np.float32)),
            "w3": np.ascontiguousarray(
                np.asarray(W3[sl], dtype=np.float32)).astype(ml_dtypes.bfloat16),
            "b1": np.ascontiguousarray(np.asarray(b1[sl], dtype=np.float32)),
            "b2": np.ascontiguousarray(np.asarray(b2[sl], dtype=np.float32)),
            "b3": np.ascontiguousarray(np.asarray(b3[sl], dtype=np.float32)),
        })
    return in_maps


def assemble(results):
    # [oC, kk, npix] -> [oC, kh, kw, n, ih, iw] -> [n, oC, ih, kh, iw, kw]
    all_out = np.concatenate([r["out"] for r in results], axis=0)
    y = all_out.reshape(OC, KH, KW, N, IH, IW)
    y = y.transpose(3, 0, 4, 1, 5, 2).reshape(N, OC, KH * IH, KW * IW)
    return np.ascontiguousarray(y)


def kernel(x, W1, b1, W2, b2, W3, b3):
    from concourse.bass_utils import run_bass_kernel_spmd

    in_maps = make_in_maps(x, W1, b1, W2, b2, W3, b3)
    nc = _get_compiled()
    res = run_bass_kernel_spmd(nc, in_maps, core_ids=list(range(NCORES)))
    return assemble(res.results)
